# revision 1
# baseline (speedup 1.0000x reference)
"""Trainium2 Bass kernel for nn_MultiHeadAttention (B=4, S=2048, D=512, H=8).

Sharding: tensor-parallel over heads — core c owns head c (Dh=64). Each core
computes q/k/v projections for its head slice (full x replicated, host-pre-
transposed to x^T in bf16), attention for its head over all 4 batches, and
the unnormalized partial out-projection O_c @ Wo[c]; the host divides each
core's partial by its softmax denominators (shipped alongside as a [B,S]
vector), sums the 8 partials, and adds the biases that commute with that
reduction (bo, bv@Wo). All on-core compute is bf16 (fp8 blows the 2e-2
error budget: each fp8-quantized operand alone contributes ~2.5%).

Engine plan (emission order IS the per-engine execution order):
  - PE: q&k projected TOGETHER (wq|wk packed on the array columns, one x
    pass), V^T projected then PE-transposed into the [key, dh] AV layout,
    row-quadrant-alternating S^T (tile_position (hb*64, 0)) so weight loads
    overlap execution, AV with the ones column of V_aug producing softmax
    denominators in PSUM row 64, out-projection.
  - ACT: exclusively exp(S/8) on [128,1024] tiles (its floor, ~140us).
  - DVE: all PSUM evacuations (no reciprocal/normalize on-core).
  - Normalization happens on host; output partial + denominators are bf16.
Attention is one flat software-pipelined stream over 128 (pair,qq,kt) steps:
per step the PE does [S^T(i+1), filler, AV(i-1)] so AV never waits on its
exp and the PE stays continuously busy (keeps the p-state at 2.4 GHz).
Batches are paired on SBUF partition halves; pair-1 prep fills PE slack
during pair-0 attention, out-projections fill during later attention.
"""
import numpy as np

import concourse.bass as bass
import concourse.mybir as mybir
import concourse.tile as tile
from concourse import bacc
from concourse.bass_utils import run_bass_kernel_spmd

B, S, D = 4, 2048, 512
H, DH = 8, 64
NCORES = 8
F32 = mybir.dt.float32
BF16 = mybir.dt.bfloat16
AF = mybir.ActivationFunctionType

NKT = S // 128          # 16 key tiles per batch
NQB = S // 512          # 4 query blocks per batch
NCH = D // 128          # 4 dm chunks

_NC_CACHE = {}


def build_kernel():
    nc = bacc.Bacc("TRN2", target_bir_lowering=False, debug=False)

    xT = nc.dram_tensor("xT", [B, D, S], BF16, kind="ExternalInput")
    # wq|wk|wv (chunk-major, 256 each) | identity (128) packed in one load
    wpack = nc.dram_tensor("wpack", [128, 896], BF16, kind="ExternalInput")
    wo = nc.dram_tensor("wo", [DH, D], BF16, kind="ExternalInput")
    bqk = nc.dram_tensor("bqk", [128, 2], F32, kind="ExternalInput")
    onesin = nc.dram_tensor("onesin", [128, 16, 2], BF16, kind="ExternalInput")
    out = nc.dram_tensor("out", [B * S, D], BF16, kind="ExternalOutput")
    dnm = nc.dram_tensor("dnm", [B, S], BF16, kind="ExternalOutput")

    with tile.TileContext(nc) as tc:
        with (
            tc.tile_pool(name="consts", bufs=1) as consts,
            tc.tile_pool(name="xtp", bufs=16) as xtp,
            tc.tile_pool(name="qkp", bufs=2) as qkp,
            tc.tile_pool(name="vtp", bufs=4) as vtp,
            tc.tile_pool(name="vp", bufs=4) as vp,
            tc.tile_pool(name="ptp", bufs=3) as ptp,
            tc.tile_pool(name="otp", bufs=3) as otp,
            tc.tile_pool(name="sop", bufs=4) as sopp,
            tc.tile_pool(name="psA", bufs=2, space="PSUM") as psA,   # pst [128,1024] f32
            tc.tile_pool(name="psO", bufs=2, space="PSUM") as psO,   # po [65,512] f32
            tc.tile_pool(name="psM", bufs=2, space="PSUM") as psM,   # misc [128,512] f32
        ):
            bqk_sb = consts.tile([128, 2], F32)
            wp_sb = consts.tile([128, 896], BF16)
            wo_sb = consts.tile([DH, D], BF16)
            warm = consts.tile([128, 1], BF16)
            nc.sync.dma_start(out=bqk_sb[:], in_=bqk[:])
            nc.scalar.dma_start(out=wp_sb[:], in_=wpack[:])
            # warmup: pulls the Exp table load (~1.3us) into the kernel head
            nc.scalar.activation(warm[:], bqk_sb[:, 0:1], AF.Exp, scale=0.125)
            bq_sb = bqk_sb[:, 0:1]
            bk_sb = bqk_sb[:, 1:2]
            ident = wp_sb[:, 768:896]

            def w_qk(ci):
                # per-chunk [wq | wk] packed side by side (128 cols)
                return wp_sb[:, bass.ds(ci * 128, 128)]

            def w_v(ci):
                return wp_sb[:, bass.ds(512 + ci * DH, DH)]

            state = {}

            def alloc_pair(pr):
                st = {"xt": {}, "vt": {}, "v": {}, "ot": {}}
                st["qt"] = qkp.tile([128, S], BF16, tag="qt", name=f"qt_{pr}")
                st["kt"] = qkp.tile([128, S], BF16, tag="kt", name=f"kt_{pr}")
                for half in range(2):
                    b = pr * 2 + half
                    st["v"][half] = vp.tile([128, NKT, DH + 2], BF16, tag="v", name=f"v_{b}")
                state[pr] = st

            def emit_ones(pr):
                for half in range(2):
                    nc.gpsimd.dma_start(
                        out=state[pr]["v"][half][:, :, DH:DH + 2], in_=onesin[:]
                    )

            def emit_xt_blk(pr, blks, engs):
                st = state[pr]
                if 0 in blks:
                    for half in range(2):
                        b = pr * 2 + half
                        st["xt"][half] = [
                            xtp.tile([128, S], BF16, tag="xt", name=f"xt_{b}_{ci}")
                            for ci in range(NCH)
                        ]
                q = 0
                for blk in blks:
                    for ci in range(NCH):
                        for half in range(2):
                            eng = engs[q % len(engs)]
                            q += 1
                            eng.dma_start(
                                out=st["xt"][half][ci][:, bass.ts(blk, 512)],
                                in_=xT[pr * 2 + half, bass.ts(ci, 128), bass.ts(blk, 512)],
                            )

            def emit_prep_qk(pr, blk, half):
                # one x pass computes q (PSUM rows 0-63) AND k (rows 64-127)
                st = state[pr]
                sl = bass.ts(blk, 512)
                pqk = psM.tile([128, 512], F32, tag="psM", name=f"pqk_{pr}_{blk}_{half}")
                for ci in range(NCH):
                    nc.tensor.matmul(
                        pqk[:], w_qk(ci), st["xt"][half][ci][:, sl],
                        start=(ci == 0), stop=(ci == NCH - 1),
                        tile_position=(0, 0),
                    )
                hsl = bass.ds(half * DH, DH)
                nc.vector.tensor_scalar_add(st["qt"][hsl, sl], pqk[0:DH, :], bqk_sb[0:DH, 0:1])
                nc.vector.tensor_scalar_add(st["kt"][hsl, sl], pqk[DH:128, :], bqk_sb[0:DH, 1:2])

            def emit_prep_v(pr, blk):
                st = state[pr]
                sl = bass.ts(blk, 512)
                if blk == 0:
                    for half in range(2):
                        b = pr * 2 + half
                        vt_b = vtp.tile([DH, S], BF16, tag="vt", name=f"vt_{b}")
                        st["vt"][half] = vt_b
                pv = psM.tile([128, 512], F32, tag="psM", name=f"pv_{pr}_{blk}")
                for ci in range(NCH):
                    for half in range(2):
                        nc.tensor.matmul(
                            pv[half * DH:(half + 1) * DH, :],
                            w_v(ci), st["xt"][half][ci][:, sl],
                            start=(ci == 0), stop=(ci == NCH - 1),
                            tile_position=(0, half * DH),
                        )
                nc.vector.tensor_copy(st["vt"][0][:, sl], pv[0:DH, :])
                nc.vector.tensor_copy(st["vt"][1][:, sl], pv[DH:128, :])
                for half in range(2):
                    b = pr * 2 + half
                    pvtr = psM.tile([128, 256], BF16, tag="psM", name=f"pvtr_{b}_{blk}")
                    for j in range(4):
                        nc.tensor.transpose(
                            pvtr[:, bass.ts(j, 64)],
                            st["vt"][half][:, bass.ds(blk * 512 + j * 128, 128)],
                            wp_sb[0:DH, bass.ds(768, DH)],
                        )
                    nc.vector.tensor_copy(
                        st["v"][half][:, bass.ds(blk * 4, 4), 0:DH],
                        pvtr[:].rearrange("p (k m) -> p k m", m=64),
                    )

            # --- software-pipelined attention over a flat (pr, qq, kt) stream:
            # per step i the PE does [S^T(i+1), filler, AV(i-1)], so AV never
            # waits on its exp (which completed during the previous step) and
            # the PE stays continuously busy (p-state ramp to 2.4 GHz).
            psts = {}
            ptts = {}
            pos = {}

            def emit_st(pr, qq, kt_i, i):
                st = state[pr]
                pst = psA.tile([128, 1024], F32, tag="psA", name=f"pst_{pr}_{qq}_{kt_i}")
                for hb in range(2):
                    nc.tensor.matmul(
                        pst[:, bass.ts(hb, 512)],
                        st["kt"][hb * DH:(hb + 1) * DH, bass.ts(kt_i, 128)],
                        st["qt"][hb * DH:(hb + 1) * DH, bass.ts(qq, 512)],
                        start=True, stop=True,
                        tile_position=(hb * DH, 0),
                    )
                psts[i] = pst

            def emit_exp(i):
                ptt = ptp.tile([128, 1024], BF16, tag="pt", name=f"ptt_{i}")
                nc.scalar.activation(ptt[:], psts.pop(i)[:], AF.Exp, scale=0.125)
                ptts[i] = ptt

            def emit_av(pr, qq, kt_i, i):
                st = state[pr]
                if kt_i == 0:
                    pos[(pr, qq)] = [
                        psO.tile([DH + 1, 512], F32, tag="psO", name=f"po{hb}_{pr}_{qq}")
                        for hb in range(2)
                    ]
                po = pos[(pr, qq)]
                ptt = ptts.pop(i)
                for hb in range(2):
                    nc.tensor.matmul(
                        po[hb][:],
                        st["v"][hb][:, kt_i, 0:DH + 1],
                        ptt[:, bass.ts(hb, 512)],
                        start=(kt_i == 0), stop=(kt_i == NKT - 1),
                    )

            def emit_po_evac(pr, qq):
                st = state[pr]
                if qq == 0:
                    for half in range(2):
                        st["ot"][half] = otp.tile(
                            [DH + 1, S], BF16, tag="ot", name=f"ot_{pr * 2 + half}"
                        )
                po = pos.pop((pr, qq))
                for hb in range(2):
                    nc.vector.tensor_copy(st["ot"][hb][:, bass.ts(qq, 512)], po[hb][:])

            def emit_op_tt(pr, half, tt, evac_eng=None):
                st = state[pr]
                b = pr * 2 + half
                ot_b = st["ot"][half]
                pop = psM.tile([128, 512], F32, tag="psM", name=f"pop_{b}_{tt}")
                nc.tensor.matmul(
                    pop[:], ot_b[0:DH, bass.ts(tt, 128)], wo_sb[:],
                    start=True, stop=True,
                )
                so = sopp.tile([128, 512], BF16, tag="so", name=f"so_{b}_{tt}")
                if evac_eng is nc.scalar:
                    nc.scalar.copy(so[:], pop[:])
                else:
                    nc.vector.tensor_copy(so[:], pop[:])
                eng = nc.gpsimd if (b * NKT + tt) % 2 == 0 else nc.sync
                eng.dma_start(
                    out=out[bass.ds(b * S + tt * 128, 128), :], in_=so[:]
                )

            def emit_dnm_dma(pr, half):
                b = pr * 2 + half
                nc.gpsimd.dma_start(
                    out=dnm[b:b + 1, :], in_=state[pr]["ot"][half][DH:DH + 1, :]
                )

            # ---------------- emission schedule ----------------
            import functools
            P = functools.partial
            alloc_pair(0)
            alloc_pair(1)
            # head: blk0 of pair 0 lands before anything else bulky
            emit_xt_blk(0, [0], [nc.sync, nc.gpsimd])
            emit_ones(0)
            emit_ones(1)
            emit_xt_blk(0, [1, 2, 3], [nc.sync, nc.gpsimd])
            nc.gpsimd.dma_start(out=wo_sb[:], in_=wo[:])
            emit_xt_blk(1, [0, 1, 2, 3], [nc.sync, nc.gpsimd])

            # minimal pair-0 head: block 0 of q/k/v (+ first 8 V transposes)
            emit_prep_qk(0, 0, 0)
            emit_prep_qk(0, 0, 1)
            emit_prep_v(0, 0)

            # fillers staged by earliest-allowed step so a filler whose DMA
            # hasn't landed can't convoy the in-order PE queue
            fill = []
            ms = 0
            for blk in (1, 2, 3):
                fill.append((ms, P(emit_prep_qk, 0, blk, 0))); ms += 1
                fill.append((ms, P(emit_prep_qk, 0, blk, 1))); ms += 1
                fill.append((ms, P(emit_prep_v, 0, blk))); ms += 1
            ms = 16
            for blk in range(NQB):
                fill.append((ms, P(emit_prep_qk, 1, blk, 0))); ms += 1
                fill.append((ms, P(emit_prep_qk, 1, blk, 1))); ms += 1
                fill.append((ms, P(emit_prep_v, 1, blk))); ms += 1

            units = [(pr, qq, kt) for pr in range(2) for qq in range(NQB)
                     for kt in range(NKT)]
            NSTEP = len(units)
            emit_st(*units[0], 0)
            for i in range(NSTEP):
                emit_exp(i)
                if i + 1 < NSTEP:
                    emit_st(*units[i + 1], i + 1)
                npop = 2 if i >= NSTEP - 16 else 1
                for _ in range(npop):
                    if fill and fill[0][0] <= i:
                        fill.pop(0)[1]()
                if i >= 1:
                    pr, qq, kt = units[i - 1]
                    emit_av(pr, qq, kt, i - 1)
                    if kt == NKT - 1:
                        emit_po_evac(pr, qq)
                        for half in range(2):
                            for tt in range(qq * 4, qq * 4 + 4):
                                fill.append((0, P(emit_op_tt, pr, half, tt)))
                        if qq == NQB - 1:
                            for half in range(2):
                                fill.append((0, P(emit_dnm_dma, pr, half)))
            pr, qq, kt = units[NSTEP - 1]
            emit_av(pr, qq, kt, NSTEP - 1)
            while fill:
                fill.pop(0)[1]()
            # tail: fine-grained evac of the last qq so each out-projection
            # starts as soon as its 128-token slice of O^T lands; ACT (done
            # with exps) takes half the final PSUM->SBUF copies off DVE
            st = state[pr]
            po = pos.pop((pr, qq))
            for tt_rel in range(4):
                tt = qq * 4 + tt_rel
                dsl = bass.ds(qq * 512 + tt_rel * 128, 128)
                for hb in range(2):
                    nc.vector.tensor_copy(
                        st["ot"][hb][:, dsl], po[hb][:, bass.ts(tt_rel, 128)]
                    )
                for hb in range(2):
                    emit_op_tt(pr, hb, tt)
            for half in range(2):
                emit_dnm_dma(pr, half)

    nc.compile()
    return nc


def kernel(x, Wq, bq, Wk, bk, Wv, bv, Wo, bo):
    import ml_dtypes
    BF = ml_dtypes.bfloat16
    x = np.asarray(x, dtype=np.float32)
    xT = np.ascontiguousarray(np.transpose(x, (0, 2, 1))).astype(BF)
    Wq = np.asarray(Wq, dtype=np.float32)
    Wk = np.asarray(Wk, dtype=np.float32)
    Wv = np.asarray(Wv, dtype=np.float32)
    Wo = np.asarray(Wo, dtype=np.float32)
    bq = np.asarray(bq, dtype=np.float32)
    bk = np.asarray(bk, dtype=np.float32)
    bv = np.asarray(bv, dtype=np.float32)
    bo = np.asarray(bo, dtype=np.float32)

    if "nc" not in _NC_CACHE:
        _NC_CACHE["nc"] = build_kernel()
    nc = _NC_CACHE["nc"]

    eye = np.eye(128, dtype=np.float32)
    ones = np.zeros((128, 16, 2), dtype=BF)
    ones[:, :, 0] = 1.0

    def cmajor(W, hs):
        # [p, c, m] = W[c*128+p, hs][m]
        return W[:, hs].reshape(4, 128, DH).transpose(1, 0, 2)

    in_maps = []
    for c in range(NCORES):
        hs = slice(c * DH, (c + 1) * DH)
        # per-chunk [wq | wk] interleave (512), then wv (256), then identity
        qk = np.concatenate([cmajor(Wq, hs), cmajor(Wk, hs)], axis=2).reshape(128, 512)
        wp = np.concatenate([qk, cmajor(Wv, hs).reshape(128, 256), eye], axis=1)
        in_maps.append({
            "xT": xT,
            "wpack": np.ascontiguousarray(wp).astype(BF),
            "wo": np.ascontiguousarray(Wo[hs, :]).astype(BF),
            "bqk": np.ascontiguousarray(
                np.stack([np.concatenate([bq[hs], bq[hs]]),
                          np.concatenate([bk[hs], bk[hs]])], axis=1)),
            "onesin": ones,
        })

    res = run_bass_kernel_spmd(nc, in_maps, list(range(NCORES)))

    acc = np.zeros((B * S, D), dtype=np.float32)
    for c in range(NCORES):
        o = np.asarray(res.results[c]["out"]).astype(np.float32)
        d = np.asarray(res.results[c]["dnm"]).astype(np.float32)
        acc += o / d.reshape(B * S, 1)
    # biases that commute with the head-reduction, applied at gather time
    acc += bo[None, :] + (bv @ Wo)[None, :]
    return acc.reshape(B, S, D)



# revision 4
# speedup vs baseline: 1.0847x; 1.0847x over previous
"""Trainium2 Bass kernel for nn_MultiHeadAttention (B=4, S=2048, D=512, H=8).

Sharding: 2D tensor x data parallel - core c = (hp=c//2, bp=c%2) owns heads
{2hp, 2hp+1} and batches {2bp, 2bp+1}. Each core computes q/k/v projections
for its two heads over its two batches (x^T shipped bf16, 4MB/core), runs
attention per (batch, head) with both heads packed on SBUF partition halves,
and ships per-head unnormalized partial out-projections plus softmax
denominators (riding row 64 of each O^T tile via the ones-column of V_aug);
the host divides by denominators, sums the 16 (core, head) partials, and adds
the commuting biases (bo, bv@Wo). All on-core compute is bf16.

Engine plan (emission order IS the per-engine execution order):
  - PE: q then k projected per 128-dim head-pair slice (full M=128), V
    projected with M=128 (both heads in one pass), PE-transposed into the
    [key, dh] AV layout, row-quadrant S^T (tile_position (h*64, 0)) so the
    two heads' score matmuls overlap, AV with per-head ones columns of V_aug
    producing softmax denominators in PSUM row 64, per-head out-projection.
  - ACT: exclusively exp(S/8) on [128,1024] tiles - the ~144us bottleneck;
    everything else is scheduled to hide under it.
  - DVE: all PSUM evacuations.
Attention is one flat software-pipelined stream over 128 (b,qq,kt) steps:
per step the PE does [S^T(i+1), filler, AV(i-1)] so AV never waits on its
exp. xT arrives as 4 small blk0 DMAs (to start compute ~2us in) plus big
[128,1536]/[128,2048] contiguous transfers for the rest.
"""
import numpy as np

import concourse.bass as bass
import concourse.mybir as mybir
import concourse.tile as tile
from concourse import bacc
from concourse.bass_utils import run_bass_kernel_spmd

B, S, D = 4, 2048, 512
H, DH = 8, 64
NCORES = 8
F32 = mybir.dt.float32
BF16 = mybir.dt.bfloat16
AF = mybir.ActivationFunctionType

NB = 2                  # local batches per core
NKT = S // 128          # 16 key tiles per batch
NQB = S // 512          # 4 query blocks per batch
NCH = D // 128          # 4 dm chunks

_NC_CACHE = {}


def build_kernel():
    nc = bacc.Bacc("TRN2", target_bir_lowering=False, debug=False)

    xT = nc.dram_tensor("xT", [NB, D, S], BF16, kind="ExternalInput")
    # per-chunk [wq(128) | wk(128)] (4*256) | wv per-chunk (4*128) | ident
    wpack = nc.dram_tensor("wpack", [128, 1664], BF16, kind="ExternalInput")
    wo = nc.dram_tensor("wo", [DH, 2 * D], BF16, kind="ExternalInput")
    bqk = nc.dram_tensor("bqk", [128, 2], F32, kind="ExternalInput")
    onesin = nc.dram_tensor("onesin", [128, NKT, 1], BF16, kind="ExternalInput")
    out = nc.dram_tensor("out", [2, NB * S, D], BF16, kind="ExternalOutput")
    dnm = nc.dram_tensor("dnm", [2, NB, S], BF16, kind="ExternalOutput")

    with tile.TileContext(nc) as tc:
        with (
            tc.tile_pool(name="consts", bufs=1) as consts,
            tc.tile_pool(name="xtp", bufs=8) as xtp,
            tc.tile_pool(name="qkp", bufs=4) as qkp,
            tc.tile_pool(name="vtp", bufs=2) as vtp,
            tc.tile_pool(name="vp", bufs=2) as vp,
            tc.tile_pool(name="ptp", bufs=3) as ptp,
            tc.tile_pool(name="otp", bufs=6) as otp,
            tc.tile_pool(name="sop", bufs=4) as sopp,
            tc.tile_pool(name="psA", bufs=2, space="PSUM") as psA,   # pst [128,1024] f32
            tc.tile_pool(name="psO", bufs=2, space="PSUM") as psO,   # po [65,512] f32
            tc.tile_pool(name="psM", bufs=2, space="PSUM") as psM,   # misc [128,512] f32
        ):
            bqk_sb = consts.tile([128, 2], F32)
            wp_sb = consts.tile([128, 1664], BF16)
            wo_sb = consts.tile([DH, 2 * D], BF16)
            warm = consts.tile([128, 1], BF16)
            nc.sync.dma_start(out=bqk_sb[:], in_=bqk[:])
            nc.scalar.dma_start(out=wp_sb[:], in_=wpack[:])
            # warmup: pulls the Exp table load (~2.7us) into the kernel head
            nc.scalar.activation(warm[:], bqk_sb[:, 0:1], AF.Exp, scale=0.125)
            ident = wp_sb[:, 1536:1664]

            def w_q(ci):
                return wp_sb[:, bass.ds(ci * 256, 128)]

            def w_k(ci):
                return wp_sb[:, bass.ds(ci * 256 + 128, 128)]

            def w_v(ci):
                return wp_sb[:, bass.ds(1024 + ci * 128, 128)]

            state = {}

            def alloc_b(b):
                st = {"xt": {}, "ot": {}}
                st["qt"] = qkp.tile([128, S], BF16, tag="qt", name=f"qt_{b}")
                st["kt"] = qkp.tile([128, S], BF16, tag="kt", name=f"kt_{b}")
                # [V_h0(0:64) | ones(64) | V_h1(65:129) | ones(129)] pad->132
                st["v"] = vp.tile([128, NKT, 132], BF16, tag="v", name=f"v_{b}")
                state[b] = st

            def emit_ones(b):
                nc.gpsimd.dma_start(out=state[b]["v"][:, :, 64:65], in_=onesin[:])
                nc.gpsimd.dma_start(out=state[b]["v"][:, :, 129:130], in_=onesin[:])

            def emit_xt_head(b, engs):
                # blk0 chunks as small DMAs so prep can start early
                st = state[b]
                st["xt"] = [
                    xtp.tile([128, S], BF16, tag="xt", name=f"xt_{b}_{ci}")
                    for ci in range(NCH)
                ]
                for ci in range(NCH):
                    engs[ci % len(engs)].dma_start(
                        out=st["xt"][ci][:, 0:512],
                        in_=xT[b, bass.ts(ci, 128), 0:512],
                    )

            def emit_xt_rest(b, engs):
                st = state[b]
                for ci in range(NCH):
                    engs[ci % len(engs)].dma_start(
                        out=st["xt"][ci][:, 512:S],
                        in_=xT[b, bass.ts(ci, 128), 512:S],
                    )

            def emit_prep_q(b, blk):
                st = state[b]
                sl = bass.ts(blk, 512)
                pq = psM.tile([128, 512], F32, tag="psM", name=f"pq_{b}_{blk}")
                for ci in range(NCH):
                    nc.tensor.matmul(
                        pq[:], w_q(ci), st["xt"][ci][:, sl],
                        start=(ci == 0), stop=(ci == NCH - 1),
                    )
                nc.vector.tensor_scalar_add(st["qt"][:, sl], pq[:], bqk_sb[:, 0:1])

            def emit_prep_k(b, blk):
                st = state[b]
                sl = bass.ts(blk, 512)
                pk = psM.tile([128, 512], F32, tag="psM", name=f"pk_{b}_{blk}")
                for ci in range(NCH):
                    nc.tensor.matmul(
                        pk[:], w_k(ci), st["xt"][ci][:, sl],
                        start=(ci == 0), stop=(ci == NCH - 1),
                    )
                nc.vector.tensor_scalar_add(st["kt"][:, sl], pk[:], bqk_sb[:, 1:2])

            def emit_prep_vmm(b, blk):
                st = state[b]
                sl = bass.ts(blk, 512)
                if blk == 0:
                    st["vt"] = vtp.tile([128, S], BF16, tag="vt", name=f"vt_{b}")
                pv = psM.tile([128, 512], F32, tag="psM", name=f"pv_{b}_{blk}")
                for ci in range(NCH):
                    nc.tensor.matmul(
                        pv[:], w_v(ci), st["xt"][ci][:, sl],
                        start=(ci == 0), stop=(ci == NCH - 1),
                    )
                nc.vector.tensor_copy(st["vt"][:, sl], pv[:])

            def emit_prep_vtr(b, blk):
                st = state[b]
                pvtr = psM.tile([128, 512], BF16, tag="psM", name=f"pvtr_{b}_{blk}")
                for j in range(4):
                    nc.tensor.transpose(
                        pvtr[:, bass.ts(j, 128)],
                        st["vt"][:, bass.ds(blk * 512 + j * 128, 128)],
                        ident,
                    )
                # [p, (j a c)] -> v[:, blk*4+j, a*65 + c] for c in 0:64
                nc.vector.tensor_copy(
                    st["v"][:, bass.ds(blk * 4, 4), 0:130]
                    .rearrange("p k (a c) -> p k a c", c=65)[:, :, :, 0:64],
                    pvtr[:].rearrange("p (k a c) -> p k a c", k=4, a=2),
                )

            # --- software-pipelined attention over a flat (b, qq, kt) stream
            psts = {}
            ptts = {}
            pos = {}

            def emit_st(b, qq, kt_i, i):
                st = state[b]
                pst = psA.tile([128, 1024], F32, tag="psA", name=f"pst_{b}_{qq}_{kt_i}")
                for h in range(2):
                    nc.tensor.matmul(
                        pst[:, bass.ts(h, 512)],
                        st["kt"][h * DH:(h + 1) * DH, bass.ts(kt_i, 128)],
                        st["qt"][h * DH:(h + 1) * DH, bass.ts(qq, 512)],
                        start=True, stop=True,
                        tile_position=(h * DH, 0),
                    )
                psts[i] = pst

            def emit_exp(i):
                ptt = ptp.tile([128, 1024], BF16, tag="pt", name=f"ptt_{i}")
                nc.scalar.activation(ptt[:], psts.pop(i)[:], AF.Exp, scale=0.125)
                ptts[i] = ptt

            def emit_av(b, qq, kt_i, i):
                st = state[b]
                if kt_i == 0:
                    pos[(b, qq)] = [
                        psO.tile([DH + 1, 512], F32, tag="psO", name=f"po{h}_{b}_{qq}")
                        for h in range(2)
                    ]
                po = pos[(b, qq)]
                ptt = ptts.pop(i)
                for h in range(2):
                    nc.tensor.matmul(
                        po[h][:],
                        st["v"][:, kt_i, bass.ds(h * 65, 65)],
                        ptt[:, bass.ts(h, 512)],
                        start=(kt_i == 0), stop=(kt_i == NKT - 1),
                    )

            def emit_po_evac(b, qq):
                st = state[b]
                if qq == 0:
                    for h in range(2):
                        st["ot"][h] = otp.tile(
                            [DH + 1, S], BF16, tag="ot", name=f"ot_{b}_{h}"
                        )
                po = pos.pop((b, qq))
                for h in range(2):
                    nc.vector.tensor_copy(st["ot"][h][:, bass.ts(qq, 512)], po[h][:])

            def emit_op_tt(b, h, tt, evac_eng=None):
                st = state[b]
                ot_h = st["ot"][h]
                pop = psM.tile([128, 512], F32, tag="psM", name=f"pop_{b}_{h}_{tt}")
                nc.tensor.matmul(
                    pop[:], ot_h[0:DH, bass.ts(tt, 128)],
                    wo_sb[:, bass.ds(h * D, D)],
                    start=True, stop=True,
                )
                so = sopp.tile([128, 512], BF16, tag="so", name=f"so_{b}_{h}_{tt}")
                if evac_eng is nc.scalar:
                    nc.scalar.copy(so[:], pop[:])
                else:
                    nc.vector.tensor_copy(so[:], pop[:])
                eng = nc.gpsimd if (b * 32 + h * NKT + tt) % 2 == 0 else nc.sync
                eng.dma_start(
                    out=out[h, bass.ds(b * S + tt * 128, 128), :], in_=so[:]
                )

            def emit_dnm_dma(b, h):
                nc.gpsimd.dma_start(
                    out=dnm[h, b:b + 1, :], in_=state[b]["ot"][h][DH:DH + 1, :]
                )

            # ---------------- emission schedule ----------------
            import functools
            P = functools.partial
            alloc_b(0)
            alloc_b(1)
            emit_xt_head(0, [nc.sync, nc.gpsimd])
            emit_ones(0)
            emit_ones(1)
            emit_xt_rest(0, [nc.sync, nc.gpsimd])
            nc.gpsimd.dma_start(out=wo_sb[:], in_=wo[:])
            emit_xt_head(1, [nc.sync, nc.gpsimd])
            emit_xt_rest(1, [nc.sync, nc.gpsimd])

            # minimal pre-stream head: blk0 of q/k/v for batch 0
            emit_prep_q(0, 0)
            emit_prep_k(0, 0)
            emit_prep_vmm(0, 0)
            emit_prep_vtr(0, 0)

            # fillers staged by earliest-allowed step; k/v before q since
            # S^T consumes all key blocks within the first 16 steps
            fill = []
            sched = [
                (1, P(emit_prep_k, 0, 1)), (2, P(emit_prep_vmm, 0, 1)),
                (3, P(emit_prep_vtr, 0, 1)),
                (4, P(emit_prep_k, 0, 2)), (5, P(emit_prep_vmm, 0, 2)),
                (6, P(emit_prep_vtr, 0, 2)),
                (7, P(emit_prep_k, 0, 3)), (8, P(emit_prep_vmm, 0, 3)),
                (9, P(emit_prep_vtr, 0, 3)),
                (10, P(emit_prep_q, 0, 1)), (11, P(emit_prep_q, 0, 2)),
                (12, P(emit_prep_q, 0, 3)),
                (16, P(emit_prep_k, 1, 0)), (18, P(emit_prep_q, 1, 0)),
                (20, P(emit_prep_vmm, 1, 0)), (22, P(emit_prep_vtr, 1, 0)),
                (24, P(emit_prep_k, 1, 1)), (26, P(emit_prep_vmm, 1, 1)),
                (28, P(emit_prep_vtr, 1, 1)),
                (30, P(emit_prep_k, 1, 2)), (32, P(emit_prep_vmm, 1, 2)),
                (34, P(emit_prep_vtr, 1, 2)),
                (36, P(emit_prep_k, 1, 3)), (38, P(emit_prep_vmm, 1, 3)),
                (40, P(emit_prep_vtr, 1, 3)),
                (42, P(emit_prep_q, 1, 1)), (44, P(emit_prep_q, 1, 2)),
                (46, P(emit_prep_q, 1, 3)),
            ]
            fill.extend(sched)

            units = [(b, qq, kt) for b in range(NB) for qq in range(NQB)
                     for kt in range(NKT)]
            NSTEP = len(units)
            emit_st(*units[0], 0)
            for i in range(NSTEP):
                emit_exp(i)
                if i + 1 < NSTEP:
                    emit_st(*units[i + 1], i + 1)
                npop = 2 if i >= NSTEP - 24 else 1
                for _ in range(npop):
                    if fill and fill[0][0] <= i:
                        fill.pop(0)[1]()
                if i >= 1:
                    b, qq, kt = units[i - 1]
                    emit_av(b, qq, kt, i - 1)
                    if kt == NKT - 1:
                        emit_po_evac(b, qq)
                        for h in range(2):
                            for tt in range(qq * 4, qq * 4 + 4):
                                fill.append((0, P(emit_op_tt, b, h, tt)))
                        if qq == NQB - 1:
                            for h in range(2):
                                fill.append((0, P(emit_dnm_dma, b, h)))
            b, qq, kt = units[NSTEP - 1]
            emit_av(b, qq, kt, NSTEP - 1)
            while fill:
                fill.pop(0)[1]()
            # tail: fine-grained evac of the last qq; ACT (done with exps)
            # takes half the final PSUM->SBUF copies off DVE
            st = state[b]
            po = pos.pop((b, qq))
            for tt_rel in range(4):
                tt = qq * 4 + tt_rel
                dsl = bass.ds(qq * 512 + tt_rel * 128, 128)
                for h in range(2):
                    nc.vector.tensor_copy(
                        st["ot"][h][:, dsl], po[h][:, bass.ts(tt_rel, 128)]
                    )
                for h in range(2):
                    emit_op_tt(b, h, tt, evac_eng=nc.scalar if h == 0 else None)
            for h in range(2):
                emit_dnm_dma(b, h)

    nc.compile()
    return nc


def kernel(x, Wq, bq, Wk, bk, Wv, bv, Wo, bo):
    import ml_dtypes
    BF = ml_dtypes.bfloat16
    x = np.asarray(x, dtype=np.float32)
    xT = np.ascontiguousarray(np.transpose(x, (0, 2, 1))).astype(BF)
    Wq = np.asarray(Wq, dtype=np.float32)
    Wk = np.asarray(Wk, dtype=np.float32)
    Wv = np.asarray(Wv, dtype=np.float32)
    Wo = np.asarray(Wo, dtype=np.float32)
    bq = np.asarray(bq, dtype=np.float32)
    bk = np.asarray(bk, dtype=np.float32)
    bv = np.asarray(bv, dtype=np.float32)
    bo = np.asarray(bo, dtype=np.float32)

    if "nc" not in _NC_CACHE:
        _NC_CACHE["nc"] = build_kernel()
    nc = _NC_CACHE["nc"]

    eye = np.eye(128, dtype=np.float32)
    ones = np.ones((128, NKT, 1), dtype=BF)

    in_maps = []
    for c in range(NCORES):
        hp, bp = c // 2, c % 2
        hs = slice(hp * 128, (hp + 1) * 128)
        qk = np.empty((128, 1024), dtype=np.float32)
        for ci in range(NCH):
            rows = slice(ci * 128, (ci + 1) * 128)
            qk[:, ci * 256:ci * 256 + 128] = Wq[rows, hs]
            qk[:, ci * 256 + 128:ci * 256 + 256] = Wk[rows, hs]
        # wv laid per-chunk [128 rows, 128 cols each]
        wv = np.concatenate(
            [Wv[ci * 128:(ci + 1) * 128, hs] for ci in range(NCH)], axis=1
        )
        wp = np.concatenate([qk, wv, eye], axis=1)
        wo2 = np.concatenate(
            [Wo[hp * 128:hp * 128 + 64, :], Wo[hp * 128 + 64:hp * 128 + 128, :]],
            axis=1,
        )
        in_maps.append({
            "xT": np.ascontiguousarray(xT[2 * bp:2 * bp + 2]),
            "wpack": np.ascontiguousarray(wp).astype(BF),
            "wo": np.ascontiguousarray(wo2).astype(BF),
            "bqk": np.ascontiguousarray(
                np.stack([bq[hs], bk[hs]], axis=1)).astype(np.float32),
            "onesin": ones,
        })

    res = run_bass_kernel_spmd(nc, in_maps, list(range(NCORES)))

    acc = np.zeros((B, S, D), dtype=np.float32)
    for c in range(NCORES):
        hp, bp = c // 2, c % 2
        o = np.asarray(res.results[c]["out"]).astype(np.float32)
        o = o.reshape(2, NB, S, D)
        d = np.asarray(res.results[c]["dnm"]).astype(np.float32)
        for h in range(2):
            for lb in range(NB):
                acc[2 * bp + lb] += o[h, lb] / d[h, lb][:, None]
    # biases that commute with the head-reduction, applied at gather time
    acc += bo[None, :] + (bv @ Wo)[None, :]
    return acc


# revision 19
# speedup vs baseline: 1.1025x; 1.0164x over previous
"""Trainium2 Bass kernel for nn_MultiHeadAttention (B=4, S=2048, D=512, H=8).

Sharding: 2D tensor x data parallel - core c = (hp=c//2, bp=c%2) owns heads
{2hp, 2hp+1} and batches {2bp, 2bp+1}. Each core computes q/k/v projections
for its two heads over its two batches (x^T shipped bf16, 4MB/core), runs
attention per (batch, head) with both heads packed on SBUF partition halves,
and ships per-head unnormalized partial out-projections plus softmax
denominators (riding row 64 of each O^T tile via the ones-column of V_aug);
the host divides by denominators, sums the 16 (core, head) partials, and adds
the commuting biases (bo, bv@Wo). All on-core compute is bf16.

Engine plan (emission order IS the per-engine execution order):
  - PE: q then k projected per 128-dim head-pair slice (full M=128), V
    projected with M=128 (both heads in one pass), PE-transposed into the
    [key, dh] AV layout, row-quadrant S^T (tile_position (h*64, 0)) so the
    two heads' score matmuls overlap, AV with per-head ones columns of V_aug
    producing softmax denominators in PSUM row 64, per-head out-projection.
  - ACT: exclusively exp(S/8) on [128,1024] tiles - the ~144us bottleneck;
    everything else is scheduled to hide under it.
  - DVE: all PSUM evacuations.
Attention is one flat software-pipelined stream over 128 (b,qq,kt) steps:
per step the PE does [S^T(i+1), filler, AV(i-1)] so AV never waits on its
exp. xT arrives as 4 small blk0 DMAs (to start compute ~2us in) plus big
[128,1536]/[128,2048] contiguous transfers for the rest.
"""
import numpy as np

import concourse.bass as bass
import concourse.mybir as mybir
import concourse.tile as tile
from concourse import bacc
from concourse.bass_utils import run_bass_kernel_spmd

B, S, D = 4, 2048, 512
H, DH = 8, 64
NCORES = 8
F32 = mybir.dt.float32
BF16 = mybir.dt.bfloat16
AF = mybir.ActivationFunctionType

NB = 2                  # local batches per core
NKT = S // 128          # 16 key tiles per batch
NQB = S // 512          # 4 query blocks per batch
NCH = D // 128          # 4 dm chunks

_NC_CACHE = {}


def build_kernel():
    nc = bacc.Bacc("TRN2", target_bir_lowering=False, debug=False)

    xT = nc.dram_tensor("xT", [NB, D, S], BF16, kind="ExternalInput")
    # per-chunk [wq(128) | wk(128)] (4*256) | wv per-chunk (4*128) | ident
    wpack = nc.dram_tensor("wpack", [128, 1664], BF16, kind="ExternalInput")
    wo = nc.dram_tensor("wo", [DH, 2 * D], BF16, kind="ExternalInput")
    bqk = nc.dram_tensor("bqk", [128, 2], F32, kind="ExternalInput")
    out = nc.dram_tensor("out", [2, NB * S, D], BF16, kind="ExternalOutput")
    dnm = nc.dram_tensor("dnm", [2, NB, S], BF16, kind="ExternalOutput")

    with tile.TileContext(nc) as tc:
        with (
            tc.tile_pool(name="consts", bufs=1) as consts,
            tc.tile_pool(name="xtp", bufs=8) as xtp,
            tc.tile_pool(name="qkp", bufs=4) as qkp,
            tc.tile_pool(name="vtp", bufs=2) as vtp,
            tc.tile_pool(name="vp", bufs=2) as vp,
            tc.tile_pool(name="ptp", bufs=4) as ptp,
            tc.tile_pool(name="otp", bufs=6) as otp,
            tc.tile_pool(name="sop", bufs=4) as sopp,
            tc.tile_pool(name="psA", bufs=2, space="PSUM") as psA,   # pst [128,1024] f32
            tc.tile_pool(name="psO", bufs=2, space="PSUM") as psO,   # po [65,512] f32
            tc.tile_pool(name="psM", bufs=2, space="PSUM") as psM,   # misc [128,512] f32
        ):
            bqk_sb = consts.tile([128, 2], F32)
            wp_sb = consts.tile([128, 1664], BF16)
            wo_sb = consts.tile([DH, 2 * D], BF16)
            warm = consts.tile([128, 1], BF16)
            nc.sync.dma_start(out=bqk_sb[:], in_=bqk[:])
            nc.scalar.dma_start(out=wp_sb[:], in_=wpack[:])
            # warmup: pulls the Exp table load (~2.7us) into the kernel head
            nc.scalar.activation(warm[:], bqk_sb[:, 0:1], AF.Exp, scale=0.125)
            ident = wp_sb[:, 1536:1664]

            def w_q(ci):
                return wp_sb[:, bass.ds(ci * 256, 128)]

            def w_k(ci):
                return wp_sb[:, bass.ds(ci * 256 + 128, 128)]

            def w_v(ci):
                return wp_sb[:, bass.ds(1024 + ci * 128, 128)]

            state = {}

            def alloc_b(b):
                st = {"xt": {}, "ot": {}}
                st["qt"] = qkp.tile([128, S], BF16, tag="qt", name=f"qt_{b}")
                st["kt"] = qkp.tile([128, S], BF16, tag="kt", name=f"kt_{b}")
                # [V_h0(0:64) | ones(64) | V_h1(65:129) | ones(129)] pad->132
                st["v"] = vp.tile([128, NKT, 132], BF16, tag="v", name=f"v_{b}")
                state[b] = st

            def emit_ones(b):
                # engine-side memset: no DMA-ring descriptors for the
                # strided ones columns of V_aug
                nc.gpsimd.memset(state[b]["v"][:, :, 64:65], 1.0)
                nc.gpsimd.memset(state[b]["v"][:, :, 129:130], 1.0)

            def emit_xt_head(b, engs):
                # blk0 chunks as small DMAs so prep can start early
                st = state[b]
                st["xt"] = [
                    xtp.tile([128, S], BF16, tag="xt", name=f"xt_{b}_{ci}")
                    for ci in range(NCH)
                ]
                for ci in range(NCH):
                    engs[ci % len(engs)].dma_start(
                        out=st["xt"][ci][:, 0:512],
                        in_=xT[b, bass.ts(ci, 128), 0:512],
                    )

            def emit_xt_rest(b, engs):
                st = state[b]
                for ci in range(NCH):
                    engs[ci % len(engs)].dma_start(
                        out=st["xt"][ci][:, 512:S],
                        in_=xT[b, bass.ts(ci, 128), 512:S],
                    )

            gst = {}

            def emit_proj_g(kind, b, blk, g):
                # 2-chunk granule of a 4-chunk projection accumulation so a
                # filler never exceeds the per-step PE budget (~2 matmuls)
                st = state[b]
                sl = bass.ts(blk, 512)
                w, bias, dst = {
                    "q": (w_q, bqk_sb[:, 0:1], "qt"),
                    "k": (w_k, bqk_sb[:, 1:2], "kt"),
                    "v": (w_v, None, None),
                }[kind]
                if g == 0:
                    gst[(kind, b, blk)] = psM.tile(
                        [128, 512], F32, tag="psM", name=f"p{kind}_{b}_{blk}"
                    )
                ps = gst[(kind, b, blk)]
                for ci in (2 * g, 2 * g + 1):
                    nc.tensor.matmul(
                        ps[:], w(ci), st["xt"][ci][:, sl],
                        start=(ci == 0), stop=(ci == NCH - 1),
                    )
                if g == 1:
                    gst.pop((kind, b, blk))
                    if kind == "v":
                        if blk == 0:
                            st["vt"] = vtp.tile(
                                [128, S], BF16, tag="vt", name=f"vt_{b}"
                            )
                        nc.vector.tensor_copy(st["vt"][:, sl], ps[:])
                    else:
                        nc.vector.tensor_scalar_add(st[dst][:, sl], ps[:], bias)

            def emit_vtr_g(b, blk, g):
                # 2 PE transposes + their evac into the AV layout
                st = state[b]
                pvtr = psM.tile([128, 256], BF16, tag="psM",
                                name=f"pvtr_{b}_{blk}_{g}")
                for jj in range(2):
                    j = 2 * g + jj
                    nc.tensor.transpose(
                        pvtr[:, bass.ts(jj, 128)],
                        st["vt"][:, bass.ds(blk * 512 + j * 128, 128)],
                        ident,
                    )
                # [p, (j a c)] -> v[:, blk*4+2g+j, a*65 + c] for c in 0:64
                nc.vector.tensor_copy(
                    st["v"][:, bass.ds(blk * 4 + 2 * g, 2), 0:130]
                    .rearrange("p k (a c) -> p k a c", c=65)[:, :, :, 0:64],
                    pvtr[:].rearrange("p (k a c) -> p k a c", k=2, a=2),
                )

            def emit_prep_q(b, blk):
                emit_proj_g("q", b, blk, 0)
                emit_proj_g("q", b, blk, 1)

            def emit_prep_k(b, blk):
                emit_proj_g("k", b, blk, 0)
                emit_proj_g("k", b, blk, 1)

            def emit_prep_vmm(b, blk):
                emit_proj_g("v", b, blk, 0)
                emit_proj_g("v", b, blk, 1)

            def emit_prep_vtr(b, blk):
                emit_vtr_g(b, blk, 0)
                emit_vtr_g(b, blk, 1)

            # --- software-pipelined attention over a flat (b, qq, kt) stream
            psts = {}
            ptts = {}
            pos = {}

            def emit_st(b, qq, kt_i, i):
                st = state[b]
                pst = psA.tile([128, 1024], F32, tag="psA", name=f"pst_{b}_{qq}_{kt_i}")
                for h in range(2):
                    nc.tensor.matmul(
                        pst[:, bass.ts(h, 512)],
                        st["kt"][h * DH:(h + 1) * DH, bass.ts(kt_i, 128)],
                        st["qt"][h * DH:(h + 1) * DH, bass.ts(qq, 512)],
                        start=True, stop=True,
                        tile_position=(h * DH, 0),
                    )
                psts[i] = pst

            def emit_exp(i):
                ptt = ptp.tile([128, 1024], BF16, tag="pt", name=f"ptt_{i}")
                nc.scalar.activation(ptt[:], psts.pop(i)[:], AF.Exp, scale=0.125)
                ptts[i] = ptt

            def emit_av(b, qq, kt_i, i):
                st = state[b]
                if kt_i == 0:
                    pos[(b, qq)] = [
                        psO.tile([DH + 1, 512], F32, tag="psO", name=f"po{h}_{b}_{qq}")
                        for h in range(2)
                    ]
                po = pos[(b, qq)]
                ptt = ptts.pop(i)
                for h in range(2):
                    nc.tensor.matmul(
                        po[h][:],
                        st["v"][:, kt_i, bass.ds(h * 65, 65)],
                        ptt[:, bass.ts(h, 512)],
                        start=(kt_i == 0), stop=(kt_i == NKT - 1),
                    )

            def emit_po_evac(b, qq):
                st = state[b]
                if qq == 0:
                    for h in range(2):
                        st["ot"][h] = otp.tile(
                            [DH + 1, S], BF16, tag="ot", name=f"ot_{b}_{h}"
                        )
                po = pos.pop((b, qq))
                for h in range(2):
                    nc.vector.tensor_copy(st["ot"][h][:, bass.ts(qq, 512)], po[h][:])

            def emit_op_tt(b, h, tt, evac_eng=None, dma_eng=None):
                st = state[b]
                ot_h = st["ot"][h]
                pop = psM.tile([128, 512], F32, tag="psM", name=f"pop_{b}_{h}_{tt}")
                nc.tensor.matmul(
                    pop[:], ot_h[0:DH, bass.ts(tt, 128)],
                    wo_sb[:, bass.ds(h * D, D)],
                    start=True, stop=True,
                )
                so = sopp.tile([128, 512], BF16, tag="so", name=f"so_{b}_{h}_{tt}")
                if evac_eng is nc.scalar:
                    nc.scalar.copy(so[:], pop[:])
                else:
                    nc.vector.tensor_copy(so[:], pop[:])
                if dma_eng is None:
                    dma_eng = nc.gpsimd if (b * 32 + h * NKT + tt) % 2 == 0 else nc.sync
                dma_eng.dma_start(
                    out=out[h, bass.ds(b * S + tt * 128, 128), :], in_=so[:]
                )

            def emit_dnm_dma(b, h):
                nc.gpsimd.dma_start(
                    out=dnm[h, b:b + 1, :], in_=state[b]["ot"][h][DH:DH + 1, :]
                )

            # ---------------- emission schedule ----------------
            import functools
            P = functools.partial
            alloc_b(0)
            alloc_b(1)
            # head: blk0 of batch 0 + weights land before anything bulky
            emit_xt_head(0, [nc.sync, nc.gpsimd])
            emit_ones(0)
            emit_ones(1)
            emit_xt_rest(0, [nc.sync, nc.gpsimd])
            nc.scalar.dma_start(out=wo_sb[:], in_=wo[:])
            emit_xt_head(1, [nc.sync, nc.gpsimd])
            emit_xt_rest(1, [nc.sync, nc.gpsimd])

            # pre-stream head: blk0 of q/k/v for batch 0 + k/vmm/q blk1
            emit_prep_q(0, 0)
            emit_prep_k(0, 0)
            emit_prep_vmm(0, 0)
            emit_prep_vtr(0, 0)
            emit_prep_k(0, 1)
            emit_prep_vmm(0, 1)
            emit_prep_q(0, 1)

            # fillers as 2-matmul granules staged by earliest-allowed step;
            # k/v before q since S^T consumes all key blocks in 16 steps
            fill = []

            def stage(s, fn, *a):
                fill.append((s, P(fn, *a)))

            stage(1, emit_vtr_g, 0, 1, 0)
            stage(2, emit_vtr_g, 0, 1, 1)
            stage(3, emit_proj_g, "k", 0, 2, 0)
            stage(4, emit_proj_g, "k", 0, 2, 1)
            stage(5, emit_proj_g, "v", 0, 2, 0)
            stage(6, emit_proj_g, "v", 0, 2, 1)
            stage(7, emit_vtr_g, 0, 2, 0)
            stage(8, emit_vtr_g, 0, 2, 1)
            stage(9, emit_proj_g, "k", 0, 3, 0)
            stage(10, emit_proj_g, "k", 0, 3, 1)
            stage(11, emit_proj_g, "v", 0, 3, 0)
            stage(12, emit_proj_g, "v", 0, 3, 1)
            stage(13, emit_vtr_g, 0, 3, 0)
            stage(14, emit_vtr_g, 0, 3, 1)
            stage(25, emit_proj_g, "q", 0, 2, 0)
            stage(26, emit_proj_g, "q", 0, 2, 1)
            stage(27, emit_proj_g, "q", 0, 3, 0)
            stage(28, emit_proj_g, "q", 0, 3, 1)
            s = 47
            for kind, blk in [("k", 0), ("q", 0), ("v", 0), ("t", 0),
                              ("k", 1), ("v", 1), ("t", 1),
                              ("k", 2), ("v", 2), ("t", 2),
                              ("k", 3), ("v", 3), ("t", 3),
                              ("q", 1), ("q", 2), ("q", 3)]:
                for g in range(2):
                    if kind == "t":
                        stage(s, emit_vtr_g, 1, blk, g)
                    else:
                        stage(s, emit_proj_g, kind, 1, blk, g)
                    s += 1

            units = [(b, qq, kt) for b in range(NB) for qq in range(NQB)
                     for kt in range(NKT)]
            NSTEP = len(units)
            emit_st(*units[0], 0)
            for i in range(NSTEP):
                emit_exp(i)
                if i + 1 < NSTEP:
                    emit_st(*units[i + 1], i + 1)
                npop = 2 if i >= NSTEP - 24 else 1
                for _ in range(npop):
                    # pop the first STAGE-READY entry in list order: staged
                    # preps (listed first) take precedence at their stage,
                    # stage-0 out-projections backfill the idle slots
                    for idx in range(len(fill)):
                        if fill[idx][0] <= i:
                            fill.pop(idx)[1]()
                            break
                    else:
                        break
                if i >= 1:
                    b, qq, kt = units[i - 1]
                    emit_av(b, qq, kt, i - 1)
                    if kt == NKT - 1:
                        emit_po_evac(b, qq)
                        for h in range(2):
                            for tt in range(qq * 4, qq * 4 + 4):
                                fill.append((0, P(emit_op_tt, b, h, tt)))
                        if qq == NQB - 1:
                            for h in range(2):
                                fill.append((0, P(emit_dnm_dma, b, h)))
            b, qq, kt = units[NSTEP - 1]
            emit_av(b, qq, kt, NSTEP - 1)
            while fill:
                fill.pop(0)[1]()
            # tail: fine-grained evac of the last qq; ACT (done with exps)
            # takes half the final PSUM->SBUF copies off DVE
            st = state[b]
            po = pos.pop((b, qq))
            tail_engs = [nc.gpsimd, nc.sync, nc.scalar]
            for tt_rel in range(4):
                tt = qq * 4 + tt_rel
                dsl = bass.ds(qq * 512 + tt_rel * 128, 128)
                for h in range(2):
                    nc.vector.tensor_copy(
                        st["ot"][h][:, dsl], po[h][:, bass.ts(tt_rel, 128)]
                    )
                for h in range(2):
                    emit_op_tt(b, h, tt, evac_eng=nc.scalar if h == 0 else None,
                               dma_eng=tail_engs[(tt_rel * 2 + h) % 3])
            for h in range(2):
                emit_dnm_dma(b, h)

    nc.compile()
    return nc


def kernel(x, Wq, bq, Wk, bk, Wv, bv, Wo, bo):
    import ml_dtypes
    BF = ml_dtypes.bfloat16
    x = np.asarray(x, dtype=np.float32)
    xT = np.ascontiguousarray(np.transpose(x, (0, 2, 1))).astype(BF)
    Wq = np.asarray(Wq, dtype=np.float32)
    Wk = np.asarray(Wk, dtype=np.float32)
    Wv = np.asarray(Wv, dtype=np.float32)
    Wo = np.asarray(Wo, dtype=np.float32)
    bq = np.asarray(bq, dtype=np.float32)
    bk = np.asarray(bk, dtype=np.float32)
    bv = np.asarray(bv, dtype=np.float32)
    bo = np.asarray(bo, dtype=np.float32)

    if "nc" not in _NC_CACHE:
        _NC_CACHE["nc"] = build_kernel()
    nc = _NC_CACHE["nc"]

    eye = np.eye(128, dtype=np.float32)

    in_maps = []
    for c in range(NCORES):
        hp, bp = c // 2, c % 2
        hs = slice(hp * 128, (hp + 1) * 128)
        qk = np.empty((128, 1024), dtype=np.float32)
        for ci in range(NCH):
            rows = slice(ci * 128, (ci + 1) * 128)
            qk[:, ci * 256:ci * 256 + 128] = Wq[rows, hs]
            qk[:, ci * 256 + 128:ci * 256 + 256] = Wk[rows, hs]
        # wv laid per-chunk [128 rows, 128 cols each]
        wv = np.concatenate(
            [Wv[ci * 128:(ci + 1) * 128, hs] for ci in range(NCH)], axis=1
        )
        wp = np.concatenate([qk, wv, eye], axis=1)
        wo2 = np.concatenate(
            [Wo[hp * 128:hp * 128 + 64, :], Wo[hp * 128 + 64:hp * 128 + 128, :]],
            axis=1,
        )
        in_maps.append({
            "xT": np.ascontiguousarray(xT[2 * bp:2 * bp + 2]),
            "wpack": np.ascontiguousarray(wp).astype(BF),
            "wo": np.ascontiguousarray(wo2).astype(BF),
            "bqk": np.ascontiguousarray(
                np.stack([bq[hs], bk[hs]], axis=1)).astype(np.float32),
        })

    res = run_bass_kernel_spmd(nc, in_maps, list(range(NCORES)))

    acc = np.zeros((B, S, D), dtype=np.float32)
    for c in range(NCORES):
        hp, bp = c // 2, c % 2
        o = np.asarray(res.results[c]["out"]).astype(np.float32)
        o = o.reshape(2, NB, S, D)
        d = np.asarray(res.results[c]["dnm"]).astype(np.float32)
        for h in range(2):
            for lb in range(NB):
                acc[2 * bp + lb] += o[h, lb] / d[h, lb][:, None]
    # biases that commute with the head-reduction, applied at gather time
    acc += bo[None, :] + (bv @ Wo)[None, :]
    return acc
